# revision 1
# baseline (speedup 1.0000x reference)
"""Trainium2 Bass kernel for nn_CryptoGNN (2-layer GCN + pooled heads).

Math notes (full derivation validated against the reference):
  With A = normalized adjacency (incl. self loops), P = [B,N] pooling matrix,
  cnt = nodes per graph:
    h1 = relu((A @ x) @ W1 + b1)
    P @ h2 = (PA @ h1) @ W2 + cnt*b2 + P @ h1        (layer 2 fully collapsed)
  where PA = P @ A is a dense [B, N] matrix computable from the integer
  graph structure alone.  Only ax = A @ x requires true sparse message
  passing on device; everything else is dense matmul.

Sharding: nodes (and the edges pointing at them) are split into 8
contiguous shards of 12544; each of the 8 NeuronCores independently
computes its shard's ax -> h1 -> partial G = [PA;P](shard)^T @ h1(shard)
([128,128]).  No collectives: the host sums the 8 partial Gs and runs the
tiny [64,*] head in numpy (microseconds).

Device phase A (per core) — sparse ax = A@x via GPSIMD ap_gather:
  * feature-transposed table: partition 16g+r holds feature r of node chunk
    g, scaled by dis[src] on device (one DVE mul)
  * ap_gather #1: per-group dst-sorted edge stream of src columns
  * fp32 prefix scan along the stream (tensor_tensor_scan)
  * ap_gather #2 of per-dst boundary columns + shifted difference
    -> per-group segment sums; folded across groups with one small
    PE matmul (selection matrix).
  +b1 folds into phase B's matmul-1 via an augmented sqrt(deg) row;
  dis[dst] folds into the host-built papt columns (relu(dis*z)=dis*relu(z)).

Device phase B (per core): 98 node tiles of 128 (papt streamed in 12-tile
slab DMAs, relu batched 4 tiles wide):
  mm1: z_t = axTaug_t @ W1aug                      [128,128] PSUM
  mm2: G += papt'_t^T @ relu(z_t)  (PSUM accumulated into one [128,128])
"""

import sys

if "/opt/trn_rl_repo" not in sys.path:
    sys.path.insert(0, "/opt/trn_rl_repo")

import numpy as np

N = 100000
E = 600000
B = 64
IN = 6
H = 128
S = 16

NSHARD = 12544            # nodes per core shard / per table chunk (98*128)
NG = 8                    # groups (= src chunks = cores)
NPAD = NSHARD * NG        # 100352
NE = NSHARD + 1           # table columns per group (+ zero column)
ND = NSHARD
NB = 12560                # boundary gather count: 1 + 12544 + 15  (%16==0)
NT = NSHARD // 128        # 98 node tiles per shard
P128 = 128

_compiled = {}


def _build_nc(JW):
    import concourse.bacc as bacc
    import concourse.mybir as mybir
    from concourse import tile

    f32 = mybir.dt.float32
    i16 = mybir.dt.int16

    nc = bacc.Bacc("TRN2", target_bir_lowering=False, debug=False)

    xt48 = nc.declare_dram_parameter("xt48", [48, NSHARD], f32, isOutput=False)
    dis_tab = nc.declare_dram_parameter("dis_tab", [NG, NE], f32, isOutput=False)
    gidx = nc.declare_dram_parameter("gidx", [P128, JW // 16], i16, isOutput=False)
    bidx = nc.declare_dram_parameter("bidx", [P128, NB // 16], i16, isOutput=False)
    sq = nc.declare_dram_parameter("sq", [1, NSHARD], f32, isOutput=False)
    papt = nc.declare_dram_parameter("papt", [NSHARD, P128], f32, isOutput=False)
    w1aug = nc.declare_dram_parameter("w1aug", [7, H], f32, isOutput=False)
    sel = nc.declare_dram_parameter("sel", [P128, 6], f32, isOutput=False)
    zrow = nc.declare_dram_parameter("zrow", [1, NE], f32, isOutput=False)
    selfsel = nc.declare_dram_parameter("selfsel", [P128, 6], f32, isOutput=False)
    gout = nc.declare_dram_parameter("gout", [P128, P128], f32, isOutput=True)

    with tile.TileContext(nc) as tc:
        with (
            tc.tile_pool(name="big", bufs=1) as big,
            tc.tile_pool(name="small", bufs=1) as small,
            tc.tile_pool(name="pstream", bufs=2) as pstream,
            tc.tile_pool(name="hbuf", bufs=3) as hbuf,
            tc.tile_pool(name="ps1", bufs=2, space="PSUM") as ps1p,
            tc.tile_pool(name="psA", bufs=2, space="PSUM") as psAp,
            tc.tile_pool(name="psG", bufs=1, space="PSUM") as psGp,
        ):
            # ---------- constants / small inputs ----------
            sel_t = small.tile([P128, 6], f32)
            nc.sync.dma_start(out=sel_t[:], in_=sel[:])
            selfsel_t = small.tile([P128, 6], f32)
            nc.sync.dma_start(out=selfsel_t[:], in_=selfsel[:])
            w1_t = small.tile([7, H], f32)
            nc.sync.dma_start(out=w1_t[:], in_=w1aug[:])
            gidx_t = small.tile([P128, JW // 16], i16)
            nc.sync.dma_start(out=gidx_t[:], in_=gidx[:])
            bidx_t = small.tile([P128, NB // 16], i16)
            nc.sync.dma_start(out=bidx_t[:], in_=bidx[:])

            # axTaug rows: 0-5 features (written by fold), 6 = sqrt(deg)
            axTaug = small.tile([7, NSHARD], f32)
            nc.sync.dma_start(out=axTaug[6:7, :], in_=sq[:])

            # ---------- phase A: table build ----------
            table = big.tile([P128, NE], f32, tag="t1")
            # zero the unused rows (r>=6 of each group) + the zero column via
            # DMA broadcasts (overlaps with the data loads; avoids a 13us
            # DVE memset on the critical path)
            for g in range(NG):
                nc.sync.dma_start(
                    out=table[16 * g + 6:16 * (g + 1), :],
                    in_=zrow[0:1, :].to_broadcast([10, NE]),
                )
            nc.vector.memset(table[:, NSHARD:NE], 0.0)
            for g in range(NG):
                nc.sync.dma_start(
                    out=table[16 * g:16 * g + 6, 0:NSHARD],
                    in_=xt48[6 * g:6 * g + 6, :],
                )
            disrep = big.tile([P128, NE], f32, tag="t2")
            for g in range(NG):
                nc.sync.dma_start(
                    out=disrep[16 * g:16 * (g + 1), :],
                    in_=dis_tab[g:g + 1, :].to_broadcast([16, NE]),
                )
            nc.vector.tensor_mul(table[:], table[:], disrep[:])

            # ---------- phase A: gather / scan / gather / diff ----------
            gath = big.tile([P128, JW], f32, tag="t3")
            nc.gpsimd.ap_gather(
                out_ap=gath[:], in_ap=table[:], idxs_ap=gidx_t[:],
                channels=P128, num_elems=NE, d=1, num_idxs=JW,
            )
            nc.vector.tensor_tensor_scan(
                out=gath[:], data0=gath[:], data1=gath[:], initial=0.0,
                op0=mybir.AluOpType.add, op1=mybir.AluOpType.bypass,
            )
            bnd = big.tile([P128, NB], f32, tag="t2")
            nc.gpsimd.ap_gather(
                out_ap=bnd[:], in_ap=gath[:], idxs_ap=bidx_t[:],
                channels=P128, num_elems=JW, d=1, num_idxs=NB,
            )
            # shifted difference, in place over bnd (writes trail reads)
            nc.vector.tensor_tensor(
                out=bnd[:, 0:ND], in0=bnd[:, 1:1 + ND], in1=bnd[:, 0:ND],
                op=mybir.AluOpType.subtract,
            )
            dt = bnd

            # ---------- phase A: fold groups (PE) -> axTaug rows 0..5 ----------
            # axT = sel^T @ dt + selfsel^T @ table   (self-loop term dis*x)
            CH = 512
            nchunks = (ND + CH - 1) // CH
            for c in range(nchunks):
                c0 = c * CH
                csz = min(CH, ND - c0)
                psA = psAp.tile([6, CH], f32, tag="psA")
                nc.tensor.matmul(
                    out=psA[:, :csz],
                    lhsT=sel_t[:],
                    rhs=dt[:, c0:c0 + csz],
                    start=True, stop=False,
                )
                nc.tensor.matmul(
                    out=psA[:, :csz],
                    lhsT=selfsel_t[:],
                    rhs=table[:, c0:c0 + csz],
                    start=False, stop=True,
                )
                nc.scalar.activation(
                    out=axTaug[0:6, c0:c0 + csz],
                    in_=psA[:, :csz],
                    func=mybir.ActivationFunctionType.Copy,
                )

            # ---------- phase B ----------
            # papt streamed as slabs of 12 node-tiles (1536 rows) per DMA.
            # dis[dst] is host-folded into papt columns, so relu needs no
            # per-partition scale and batches 4 node tiles wide.
            SLAB = 12
            QB = 4
            G_ps = psGp.tile([P128, P128], f32, tag="G")
            for s0 in range(0, NT, SLAB):
                ntiles = min(SLAB, NT - s0)
                r0 = s0 * 128
                nrows = ntiles * 128
                slab = pstream.tile([P128, SLAB * P128], f32, tag="papt")
                nc.sync.dma_start(
                    out=slab[:, 0:ntiles * P128].rearrange(
                        "p (u j) -> p u j", j=P128
                    ),
                    in_=papt[r0:r0 + nrows, :].rearrange(
                        "(u p) j -> p u j", p=128
                    ),
                )
                for q in range(0, ntiles, QB):
                    m = min(QB, ntiles - q)
                    ps1 = ps1p.tile([P128, QB * H], f32, tag="ps1")
                    for u in range(m):
                        t0 = (s0 + q + u) * 128
                        nc.tensor.matmul(
                            out=ps1[:, u * H:(u + 1) * H],
                            lhsT=axTaug[0:7, t0:t0 + 128],
                            rhs=w1_t[:],
                            start=True, stop=True,
                        )
                    h1 = hbuf.tile([P128, QB * H], f32, tag="h1")
                    nc.scalar.activation(
                        out=h1[:, :m * H], in_=ps1[:, :m * H],
                        func=mybir.ActivationFunctionType.Relu,
                    )
                    for u in range(m):
                        t = s0 + q + u
                        nc.tensor.matmul(
                            out=G_ps[:],
                            lhsT=slab[:, (q + u) * P128:(q + u + 1) * P128],
                            rhs=h1[:, u * H:(u + 1) * H],
                            start=(t == 0), stop=(t == NT - 1),
                        )

            G_sb = small.tile([P128, P128], f32)
            nc.scalar.activation(
                out=G_sb[:], in_=G_ps[:],
                func=mybir.ActivationFunctionType.Copy,
            )
            nc.sync.dma_start(out=gout[:], in_=G_sb[:])

    nc.compile()
    return nc


def _preprocess(x, edge_index, batch_idx):
    """Host-side integer/structure preprocessing. Returns per-core input maps
    (minus the device-computed parts) and head constants."""
    src = np.asarray(edge_index[0], dtype=np.int64)
    dst = np.asarray(edge_index[1], dtype=np.int64)
    loop = np.arange(N, dtype=np.int64)
    src2 = np.concatenate([src, loop])
    dst2 = np.concatenate([dst, loop])

    deg = np.bincount(dst2, minlength=N).astype(np.float32)  # >= 1
    dis = (1.0 / np.sqrt(deg)).astype(np.float32)
    sqdeg = np.sqrt(deg).astype(np.float32)

    bi = np.asarray(batch_idx, dtype=np.int64)
    cnt = np.bincount(bi, minlength=B).astype(np.float32)

    dis_pad = np.zeros(NPAD, np.float32)
    dis_pad[:N] = dis

    # dense PA = P @ A  [B, NPAD]
    w = (dis[src2] * dis[dst2]).astype(np.float64)
    flat = bi[dst2] * NPAD + src2
    PA = np.bincount(flat, weights=w, minlength=B * NPAD)
    PA = PA.reshape(B, NPAD).astype(np.float32)
    # pooling matrix P [B, NPAD]
    Pm = np.zeros((B, NPAD), np.float32)
    Pm[bi, np.arange(N)] = 1.0
    # fold dis[dst] into the pooled matrix columns: G uses relu(z) with
    # h1 = dis*relu(z), so papt rows get scaled by dis (exact: dis > 0).
    papt_full = (np.concatenate([PA, Pm], axis=0)
                 * dis_pad[None, :]).T.copy()  # [NPAD, 128]

    # per-(core, group) dst-sorted streams — REAL edges only; the appended
    # self-loops are handled analytically on device (dis^2 * x term).
    core = dst // NSHARD
    grp = src // NSHARD
    src_local = (src - grp * NSHARD).astype(np.int64)
    dst_local = (dst - core * NSHARD).astype(np.int64)
    cell = core * NG + grp
    key = cell * NSHARD + dst_local
    order = np.argsort(key, kind="stable")
    cell_s = cell[order]
    srcl_s = src_local[order]
    dstl_s = dst_local[order]
    cellcnt = np.bincount(cell_s, minlength=NG * NG)
    Jmax = int(cellcnt.max())
    JW = ((Jmax + 1 + 15) // 16) * 16
    assert JW <= 32768, JW

    cell_starts = np.zeros(NG * NG + 1, np.int64)
    np.cumsum(cellcnt, out=cell_starts[1:])

    gidx_all = np.full((NG, P128, JW // 16), NSHARD, np.int16)
    bidx_all = np.zeros((NG, P128, NB // 16), np.int16)
    for k in range(NG):
        for g in range(NG):
            ci = k * NG + g
            s0, s1 = cell_starts[ci], cell_starts[ci + 1]
            stream = np.full(JW, NSHARD, np.int64)
            stream[1:1 + (s1 - s0)] = srcl_s[s0:s1]
            gidx_all[k, 16 * g:16 * (g + 1)] = (
                stream.reshape(JW // 16, 16).T.astype(np.int16)
            )
            cnts = np.bincount(dstl_s[s0:s1], minlength=ND)
            bnd = np.cumsum(cnts)
            blist = np.zeros(NB, np.int64)
            blist[1:1 + ND] = bnd
            bidx_all[k, 16 * g:16 * (g + 1)] = (
                blist.reshape(NB // 16, 16).T.astype(np.int16)
            )

    # table-side constants
    x_np = np.asarray(x, dtype=np.float32)
    xt48 = np.zeros((48, NSHARD), np.float32)
    for g in range(NG):
        n0 = g * NSHARD
        n1 = min(n0 + NSHARD, N)
        if n1 > n0:
            xt48[6 * g:6 * g + 6, 0:n1 - n0] = x_np[n0:n1].T
    dis_tab = np.zeros((NG, NE), np.float32)
    dis_tab[:, :NSHARD] = dis_pad.reshape(NG, NSHARD)

    sq_pad = np.zeros(NPAD, np.float32)
    sq_pad[:N] = sqdeg

    sel = np.zeros((P128, 6), np.float32)
    for g in range(NG):
        for r in range(6):
            sel[16 * g + r, r] = 1.0
    # per-core self-loop selection: core k picks rows 16k+r of full=dis^2*x
    selfsel = np.zeros((NG, P128, 6), np.float32)
    for k in range(NG):
        for r in range(6):
            selfsel[k, 16 * k + r, r] = 1.0

    return {
        "JW": JW,
        "xt48": xt48,
        "dis_tab": dis_tab,
        "gidx_all": gidx_all,
        "bidx_all": bidx_all,
        "sq_pad": sq_pad,
        "papt_full": papt_full,
        "sel": sel,
        "selfsel": selfsel,
        "cnt": cnt,
    }


def _head(G, cnt, inputs):
    f = np.float32
    W2 = np.asarray(inputs["W2"], f)
    b2 = np.asarray(inputs["b2"], f)
    Wg = np.asarray(inputs["Wg"], f)
    bg = np.asarray(inputs["bg"], f)
    Et = np.asarray(inputs["Et"], f)
    Ek = np.asarray(inputs["Ek"], f)
    Ev = np.asarray(inputs["Ev"], f)
    Wp = np.asarray(inputs["Wp"], f)
    bp = np.asarray(inputs["bp"], f)
    Ekid = np.asarray(inputs["Ekid"], f)
    Wc = np.asarray(inputs["Wc"], f)
    bc = np.asarray(inputs["bc"], f)
    Wl = np.asarray(inputs["Wl"], f)
    bl = np.asarray(inputs["bl"], f)
    Wm1 = np.asarray(inputs["Wm1"], f)
    bm1 = np.asarray(inputs["bm1"], f)
    Wm2 = np.asarray(inputs["Wm2"], f)
    bm2 = np.asarray(inputs["bm2"], f)
    st = np.asarray(inputs["sol_type_idx"], np.int64)
    sk = np.asarray(inputs["sol_key_idx"], np.int64)
    sv = np.asarray(inputs["sol_val_idx"], np.int64)
    kid = np.asarray(inputs["kernel_id"], np.int64)
    cond = np.asarray(inputs["cond_vec"], f)
    loc = np.asarray(inputs["local_feats"], f)

    relu = lambda a: np.maximum(a, 0.0).astype(f)

    Ph2 = G[:B] @ W2 + cnt[:, None] * b2[None, :] + G[B:]
    g = (Ph2 / np.maximum(cnt, 1.0)[:, None]) @ Wg + bg

    seq_mean = np.concatenate(
        [Et[st].mean(axis=1), Ek[sk].mean(axis=1), Ev[sv].mean(axis=1)], axis=-1
    ).astype(f)
    p = relu(seq_mean @ Wp + bp)
    kvec = Ekid[kid]
    c = relu(cond @ Wc + bc)
    l = relu(loc @ Wl + bl)
    xf = np.concatenate([g, p, kvec, c, l], axis=1).astype(f)
    return (relu(xf @ Wm1 + bm1) @ Wm2 + bm2).astype(f)


def kernel(**inputs) -> np.ndarray:
    from concourse.bass_utils import run_bass_kernel_spmd

    pre = _preprocess(inputs["x"], inputs["edge_index"], inputs["batch_idx"])
    JW = pre["JW"]

    if JW not in _compiled:
        W1 = np.asarray(inputs["W1"], np.float32)
        b1 = np.asarray(inputs["b1"], np.float32)
        _compiled[JW] = _build_nc(JW)
    nc = _compiled[JW]

    W1 = np.asarray(inputs["W1"], np.float32)
    b1 = np.asarray(inputs["b1"], np.float32)
    w1aug = np.concatenate([W1, b1[None, :]], axis=0).astype(np.float32)  # [7,H]

    in_maps = []
    for k in range(NG):
        n0 = k * NSHARD
        in_maps.append({
            "xt48": pre["xt48"],
            "dis_tab": pre["dis_tab"],
            "gidx": pre["gidx_all"][k],
            "bidx": pre["bidx_all"][k],
            "sq": pre["sq_pad"][None, n0:n0 + NSHARD],
            "papt": np.ascontiguousarray(pre["papt_full"][n0:n0 + NSHARD]),
            "w1aug": w1aug,
            "sel": pre["sel"],
            "selfsel": pre["selfsel"][k],
            "zrow": np.zeros((1, NE), np.float32),
        })

    res = run_bass_kernel_spmd(nc, in_maps, core_ids=list(range(NG)))
    G = np.zeros((P128, P128), np.float64)
    for r in res.results:
        G += r["gout"].astype(np.float64)
    G = G.astype(np.float32)

    return _head(G, pre["cnt"], inputs)



# revision 3
# speedup vs baseline: 3.1678x; 3.1678x over previous
"""Trainium2 Bass kernel for nn_CryptoGNN (2-layer GCN + pooled heads).

Math (same collapse as validated baseline):
  With A = normalized adjacency (incl. self loops), P = [B,N] pooling,
  u[d] = sum_{s->d} dis[s]x[s] + dis[d]x[d],  zhat = u@W1 + sq*b1,
  h1 = dis*relu(zhat) = dis*relu(zhat);  relu commutes with dis>0 so
  G = [PA;P]^T-pooled h1 uses papt columns pre-scaled by dis and
  h1hat = relu(zhat) on device.  Layer 2 + heads collapse to the host
  (tiny [64,*] math).

Per-core device pipeline (8-way node sharding, 12544 dst nodes/core):
  1. bf16 compacted src table DMA ([128, TW]: 8 banks x 6 feature rows,
     only srcs with >=1 edge into this core; all 128 rows host-written
     so no SBUF garbage can reach the PE - 0*NaN != 0 on TRN2 PE)
  2. expand bf16->fp32 split across Act/DVE (gather needs 4B elems)
  3. one GPSIMD ap_gather of all dst-sorted per-bank edge streams
  4. per dst-chunk (7 chunks x 1792 dsts): fp32 prefix scan (DVE),
     boundary ap_gather (GPSIMD), shifted diff -> dt bf16 (DVE 2x mode)
  5. per node tile: z = dt_t^T @ selW + aug_t^T @ w1aug   (bf16 PE,
     selW = bank-scattered W1 rows; fold+mm1 fused, no PSUM->SBUF hop)
  6. relu -> h1 bf16 (Act), G^T += h1_t^T @ papt_t accumulated in one
     [128, 80] PSUM across all 98 tiles; single gout DMA.
Host sums the 8 partial G^T and runs the small head in numpy.
"""

import sys

if "/opt/trn_rl_repo" not in sys.path:
    sys.path.insert(0, "/opt/trn_rl_repo")

import numpy as np
import ml_dtypes

N = 100000
E = 600000
B = 64
IN = 6
H = 128
S = 16

NG = 8                    # banks (= src chunks) and cores
NS = 12544                # nodes per core shard (98*128)
NPAD = NS * NG            # 100352
NT = 98                   # node tiles per shard
C = 7                     # dst chunks per core
TPC = NT // C             # 14 tiles per chunk
NDC = NS // C             # 1792 dsts per chunk
NBC = NDC + 32            # boundary gather width per chunk (32: keeps the
                          # int16 idx slice offsets 4-byte aligned per chunk)
PCOL = 80                 # papt columns: 64 PA + <=16 local P
P128 = 128

_compiled = {}


def _build_nc(TW, JWS):
    import concourse.bacc as bacc
    import concourse.mybir as mybir
    from concourse import tile

    f32 = mybir.dt.float32
    bf16 = mybir.dt.bfloat16
    i16 = mybir.dt.int16

    JWT = sum(JWS)
    OFF = np.concatenate([[0], np.cumsum(JWS)]).astype(int)

    nc = bacc.Bacc("TRN2", target_bir_lowering=False, debug=False)

    xt = nc.declare_dram_parameter("xt", [P128, TW], bf16, isOutput=False)
    gidx = nc.declare_dram_parameter("gidx", [P128, JWT // 16], i16, isOutput=False)
    bidx = nc.declare_dram_parameter("bidx", [P128, C * NBC // 16], i16, isOutput=False)
    aug = nc.declare_dram_parameter("aug", [7, NS], bf16, isOutput=False)
    selw = nc.declare_dram_parameter("selw", [P128, H], bf16, isOutput=False)
    w1aug = nc.declare_dram_parameter("w1aug", [7, H], bf16, isOutput=False)
    papt = nc.declare_dram_parameter("papt", [NT * P128, PCOL], bf16, isOutput=False)
    gout = nc.declare_dram_parameter("gout", [P128, PCOL], f32, isOutput=True)

    with tile.TileContext(nc) as tc:
        with (
            tc.tile_pool(name="big", bufs=1) as big,
            tc.tile_pool(name="small", bufs=1) as small,
            tc.tile_pool(name="bndp", bufs=2) as bndp,
            tc.tile_pool(name="hbuf", bufs=3) as hbuf,
            tc.tile_pool(name="psz", bufs=2, space="PSUM") as pszp,
            tc.tile_pool(name="psG", bufs=1, space="PSUM") as psGp,
        ):
            # ---------- loads (xt first: it gates the critical path) ----------
            xt_t = big.tile([P128, TW], bf16, tag="xtb")
            nc.sync.dma_start(out=xt_t[:], in_=xt[:])
            gidx_t = small.tile([P128, JWT // 16], i16)
            nc.sync.dma_start(out=gidx_t[:], in_=gidx[:])
            bidx_t = small.tile([P128, C * NBC // 16], i16)
            nc.sync.dma_start(out=bidx_t[:], in_=bidx[:])
            selw_t = small.tile([P128, H], bf16)
            nc.sync.dma_start(out=selw_t[:], in_=selw[:])
            w1_t = small.tile([7, H], bf16)
            nc.sync.dma_start(out=w1_t[:], in_=w1aug[:])
            aug_t = small.tile([7, NS], bf16)
            nc.sync.dma_start(out=aug_t[:], in_=aug[:])
            papt_t = big.tile([P128, NT * PCOL], bf16, tag="papt")
            for c in range(C):
                r0 = c * TPC * P128
                nc.sync.dma_start(
                    out=papt_t[:, c * TPC * PCOL:(c + 1) * TPC * PCOL].rearrange(
                        "p (u j) -> p u j", j=PCOL
                    ),
                    in_=papt[r0:r0 + TPC * P128, :].rearrange(
                        "(u p) j -> p u j", p=P128
                    ),
                )

            # ---------- expand table to fp32 (split Act / DVE) ----------
            table = big.tile([P128, TW], f32, tag="table")
            XA = int(TW * 0.385) & ~1
            nc.scalar.activation(
                out=table[:, 0:XA], in_=xt_t[:, 0:XA],
                func=mybir.ActivationFunctionType.Copy,
            )
            nc.vector.tensor_copy(out=table[:, XA:TW], in_=xt_t[:, XA:TW])

            # ---------- gather all streams ----------
            gath = big.tile([P128, JWT], f32, tag="gath")
            nc.gpsimd.ap_gather(
                out_ap=gath[:], in_ap=table[:], idxs_ap=gidx_t[:],
                channels=P128, num_elems=TW, d=1, num_idxs=JWT,
            )

            # ---------- per-chunk scan (in place) ----------
            for c in range(C):
                o0, o1 = int(OFF[c]), int(OFF[c + 1])
                nc.vector.tensor_tensor_scan(
                    out=gath[:, o0:o1], data0=gath[:, o0:o1], data1=gath[:, o0:o1],
                    initial=0.0, op0=mybir.AluOpType.add,
                    op1=mybir.AluOpType.bypass,
                )

            # ---------- per-chunk boundary gather ----------
            bnds = []
            for c in range(C):
                o0, o1 = int(OFF[c]), int(OFF[c + 1])
                bnd = bndp.tile([P128, NBC], f32, tag=f"bnd{c % 2}")
                nc.gpsimd.ap_gather(
                    out_ap=bnd[:], in_ap=gath[:, o0:o1],
                    idxs_ap=bidx_t[:, c * NBC // 16:(c + 1) * NBC // 16],
                    channels=P128, num_elems=int(JWS[c]), d=1, num_idxs=NBC,
                )
                bnds.append(bnd)

            # ---------- per-chunk diff -> dt (bf16) ----------
            dt = big.tile([P128, NS], bf16, tag="dt")
            for c in range(C):
                d0 = c * NDC
                nc.vector.tensor_tensor(
                    out=dt[:, d0:d0 + NDC],
                    in0=bnds[c][:, 1:1 + NDC], in1=bnds[c][:, 0:NDC],
                    op=mybir.AluOpType.subtract,
                )

            # ---------- phase B: z -> relu -> G ----------
            # batches of 4 node tiles; chunk = 14 tiles = batches (4,4,4,2)
            G_ps = psGp.tile([P128, PCOL], f32, tag="G")
            QB = 4
            batches = []
            for c in range(C):
                t = c * TPC
                for sz in (4, 4, 4, 2):
                    batches.append((t, sz))
                    t += sz
            for bi, (t0, m) in enumerate(batches):
                ps = pszp.tile([P128, QB * H], f32, tag="z")
                for u in range(m):
                    n0 = (t0 + u) * P128
                    nc.tensor.matmul(
                        out=ps[:, u * H:(u + 1) * H],
                        lhsT=dt[:, n0:n0 + P128], rhs=selw_t[:],
                        start=True, stop=False,
                    )
                    nc.tensor.matmul(
                        out=ps[:, u * H:(u + 1) * H],
                        lhsT=aug_t[:, n0:n0 + P128], rhs=w1_t[:],
                        start=False, stop=True,
                    )
                h1 = hbuf.tile([P128, QB * H], bf16, tag="h1")
                nc.scalar.activation(
                    out=h1[:, :m * H], in_=ps[:, :m * H],
                    func=mybir.ActivationFunctionType.Relu,
                )
                for u in range(m):
                    t = t0 + u
                    nc.tensor.matmul(
                        out=G_ps[:],
                        lhsT=h1[:, u * H:(u + 1) * H],
                        rhs=papt_t[:, t * PCOL:(t + 1) * PCOL],
                        start=(t == 0), stop=(t == NT - 1),
                    )

            G_sb = small.tile([P128, PCOL], f32)
            nc.scalar.activation(
                out=G_sb[:], in_=G_ps[:],
                func=mybir.ActivationFunctionType.Copy,
            )
            nc.sync.dma_start(out=gout[:], in_=G_sb[:])

    nc.compile()
    return nc


def _preprocess(x, edge_index, batch_idx):
    """Integer/structure preprocessing -> per-core device inputs."""
    src = np.asarray(edge_index[0], dtype=np.int64)
    dst = np.asarray(edge_index[1], dtype=np.int64)

    deg = (np.bincount(dst, minlength=N) + 1).astype(np.float32)
    dis = (1.0 / np.sqrt(deg)).astype(np.float32)
    sq = np.sqrt(deg).astype(np.float32)
    dis_pad = np.zeros(NPAD, np.float32)
    dis_pad[:N] = dis
    sq_pad = np.zeros(NPAD, np.float32)
    sq_pad[:N] = sq

    bi = np.asarray(batch_idx, dtype=np.int64)
    cnt = np.bincount(bi, minlength=B).astype(np.float32)

    x_np = np.asarray(x, dtype=np.float32)
    x_pad = np.zeros((NPAD, IN), np.float32)
    x_pad[:N] = x_np
    disx = x_pad * dis_pad[:, None]          # [NPAD, 6]

    # ---- pooling matrices (dense PA = P @ A) ----
    loop = np.arange(N, dtype=np.int64)
    src2 = np.concatenate([src, loop])
    dst2 = np.concatenate([dst, loop])
    w = (dis[src2] * dis[dst2]).astype(np.float64)
    flat = bi[dst2] * NPAD + src2
    PA = np.bincount(flat, weights=w, minlength=B * NPAD).reshape(B, NPAD)
    PA = PA.astype(np.float32)
    Pm = np.zeros((B, NPAD), np.float32)
    Pm[bi, np.arange(N)] = 1.0
    papt_full = (np.concatenate([PA, Pm], axis=0) * dis_pad[None, :]).T  # [NPAD,128]

    # graph span per core (for the P columns)
    first_graph = np.zeros(NG, np.int64)
    span = np.zeros(NG, np.int64)
    for k in range(NG):
        lo, hi = k * NS, min((k + 1) * NS, N)
        if lo >= N:
            first_graph[k] = B - 1
            span[k] = 1
            continue
        gset = bi[lo:hi]
        first_graph[k] = gset[0]
        span[k] = gset[-1] - gset[0] + 1
        assert span[k] <= PCOL - B, f"graph span {span[k]} > {PCOL - B}"

    # ---- per (core, bank) compacted streams ----
    core = dst // NS
    bank = src // NS
    src_local = src - bank * NS
    dst_local = dst - core * NS
    chunk = dst_local // NDC
    key = ((core * NG + bank) * C + chunk) * NS + dst_local
    order = np.argsort(key, kind="stable")
    core_s = core[order]
    bank_s = bank[order]
    chunk_s = chunk[order]
    srcl_s = src_local[order]
    dstl_s = dst_local[order]

    cell = (core_s * NG + bank_s) * C + chunk_s
    cellcnt = np.bincount(cell, minlength=NG * NG * C)
    cell_starts = np.zeros(NG * NG * C + 1, np.int64)
    np.cumsum(cellcnt, out=cell_starts[1:])

    # compact column maps per (core, bank)
    colmaps = {}
    ncols = np.zeros((NG, NG), np.int64)
    for k in range(NG):
        for g in range(NG):
            s0 = cell_starts[(k * NG + g) * C]
            s1 = cell_starts[(k * NG + g + 1) * C]
            uniq = np.unique(srcl_s[s0:s1])
            colmaps[(k, g)] = uniq
            ncols[k, g] = len(uniq)
    TW = int(ncols.max()) + 1
    TW = (TW + 15) & ~15

    # per-chunk stream widths (shared across cores for one compiled NEFF)
    cc = cellcnt.reshape(NG, NG, C)
    JWS = []
    for c in range(C):
        m = int(cc[:, :, c].max())
        JWS.append(((m + 1 + 15) // 16) * 16)
    JWT = sum(JWS)
    OFF = np.concatenate([[0], np.cumsum(JWS)]).astype(int)

    # build tables / idx arrays per core
    xt_all = np.zeros((NG, P128, TW), ml_dtypes.bfloat16)
    gidx_all = np.zeros((NG, P128, JWT // 16), np.int16)
    bidx_all = np.zeros((NG, P128, C * NBC // 16), np.int16)

    for k in range(NG):
        for g in range(NG):
            uniq = colmaps[(k, g)]
            n0 = g * NS
            xt_all[k, 16 * g:16 * g + 6, 1:1 + len(uniq)] = (
                disx[n0 + uniq].T.astype(ml_dtypes.bfloat16)
            )
            # remap this (core, bank)'s stream srcs to compact cols
            s0 = cell_starts[(k * NG + g) * C]
            s1 = cell_starts[(k * NG + g + 1) * C]
            comp = np.searchsorted(uniq, srcl_s[s0:s1]) + 1

            for c in range(C):
                c0 = cell_starts[(k * NG + g) * C + c]
                c1 = cell_starts[(k * NG + g) * C + c + 1]
                ncell = c1 - c0
                stream = np.zeros(JWS[c], np.int64)
                stream[1:1 + ncell] = comp[c0 - s0:c1 - s0]
                blk = stream.reshape(JWS[c] // 16, 16).T.astype(np.int16)
                gidx_all[k, 16 * g:16 * (g + 1), OFF[c] // 16:OFF[c + 1] // 16] = blk

                dloc = dstl_s[c0:c1] - c * NDC
                cnts = np.bincount(dloc, minlength=NDC)
                blist = np.zeros(NBC, np.int64)
                np.cumsum(cnts, out=blist[1:1 + NDC])
                blist[1 + NDC:] = blist[NDC]
                bblk = blist.reshape(NBC // 16, 16).T.astype(np.int16)
                bidx_all[k, 16 * g:16 * (g + 1),
                         c * NBC // 16:(c + 1) * NBC // 16] = bblk

    # aug rows: 0-5 dis*x own chunk, 6 sq
    aug_all = np.zeros((NG, 7, NS), ml_dtypes.bfloat16)
    for k in range(NG):
        n0 = k * NS
        aug_all[k, 0:6] = disx[n0:n0 + NS].T.astype(ml_dtypes.bfloat16)
        aug_all[k, 6] = sq_pad[n0:n0 + NS].astype(ml_dtypes.bfloat16)

    # papt per core: 64 PA cols + local P cols, blocked [NT*128, PCOL]
    papt_all = np.zeros((NG, NT * P128, PCOL), ml_dtypes.bfloat16)
    for k in range(NG):
        n0 = k * NS
        pk = np.zeros((NS, PCOL), np.float32)
        pk[:, :B] = papt_full[n0:n0 + NS, :B]
        b0, sp = first_graph[k], span[k]
        pk[:, B:B + sp] = papt_full[n0:n0 + NS, B + b0:B + b0 + sp]
        papt_all[k] = pk.astype(ml_dtypes.bfloat16)

    return {
        "JW": (TW, tuple(JWS)),
        "TW": TW,
        "JWS": JWS,
        "xt_all": xt_all,
        "gidx_all": gidx_all,
        "bidx_all": bidx_all,
        "aug_all": aug_all,
        "papt_all": papt_all,
        "first_graph": first_graph,
        "span": span,
        "cnt": cnt,
    }


def _head(G, cnt, inputs):
    f = np.float32
    W2 = np.asarray(inputs["W2"], f)
    b2 = np.asarray(inputs["b2"], f)
    Wg = np.asarray(inputs["Wg"], f)
    bg = np.asarray(inputs["bg"], f)
    Et = np.asarray(inputs["Et"], f)
    Ek = np.asarray(inputs["Ek"], f)
    Ev = np.asarray(inputs["Ev"], f)
    Wp = np.asarray(inputs["Wp"], f)
    bp = np.asarray(inputs["bp"], f)
    Ekid = np.asarray(inputs["Ekid"], f)
    Wc = np.asarray(inputs["Wc"], f)
    bc = np.asarray(inputs["bc"], f)
    Wl = np.asarray(inputs["Wl"], f)
    bl = np.asarray(inputs["bl"], f)
    Wm1 = np.asarray(inputs["Wm1"], f)
    bm1 = np.asarray(inputs["bm1"], f)
    Wm2 = np.asarray(inputs["Wm2"], f)
    bm2 = np.asarray(inputs["bm2"], f)
    st = np.asarray(inputs["sol_type_idx"], np.int64)
    sk = np.asarray(inputs["sol_key_idx"], np.int64)
    sv = np.asarray(inputs["sol_val_idx"], np.int64)
    kid = np.asarray(inputs["kernel_id"], np.int64)
    cond = np.asarray(inputs["cond_vec"], f)
    loc = np.asarray(inputs["local_feats"], f)

    relu = lambda a: np.maximum(a, 0.0).astype(f)

    Ph2 = G[:B] @ W2 + cnt[:, None] * b2[None, :] + G[B:]
    g = (Ph2 / np.maximum(cnt, 1.0)[:, None]) @ Wg + bg

    seq_mean = np.concatenate(
        [Et[st].mean(axis=1), Ek[sk].mean(axis=1), Ev[sv].mean(axis=1)], axis=-1
    ).astype(f)
    p = relu(seq_mean @ Wp + bp)
    kvec = Ekid[kid]
    c = relu(cond @ Wc + bc)
    l = relu(loc @ Wl + bl)
    xf = np.concatenate([g, p, kvec, c, l], axis=1).astype(f)
    return (relu(xf @ Wm1 + bm1) @ Wm2 + bm2).astype(f)


def kernel(**inputs) -> np.ndarray:
    from concourse.bass_utils import run_bass_kernel_spmd

    pre = _preprocess(inputs["x"], inputs["edge_index"], inputs["batch_idx"])
    sig = pre["JW"]
    if sig not in _compiled:
        _compiled[sig] = _build_nc(pre["TW"], tuple(pre["JWS"]))
    nc = _compiled[sig]

    W1 = np.asarray(inputs["W1"], np.float32)
    b1 = np.asarray(inputs["b1"], np.float32)
    w1aug = np.concatenate([W1, b1[None, :]], axis=0).astype(ml_dtypes.bfloat16)
    selw = np.zeros((P128, H), ml_dtypes.bfloat16)
    for g in range(NG):
        selw[16 * g:16 * g + 6] = W1.astype(ml_dtypes.bfloat16)

    in_maps = []
    for k in range(NG):
        in_maps.append({
            "xt": pre["xt_all"][k],
            "gidx": pre["gidx_all"][k],
            "bidx": pre["bidx_all"][k],
            "aug": pre["aug_all"][k],
            "selw": selw,
            "w1aug": w1aug,
            "papt": pre["papt_all"][k],
        })

    res = run_bass_kernel_spmd(nc, in_maps, core_ids=list(range(NG)))

    Gpa = np.zeros((B, H), np.float64)
    Gp = np.zeros((B, H), np.float64)
    for k, r in enumerate(res.results):
        gt = r["gout"].astype(np.float64)      # [128 f, 80 c]
        Gpa += gt[:, :B].T
        b0, sp = pre["first_graph"][k], pre["span"][k]
        Gp[b0:b0 + sp] += gt[:, B:B + sp].T
    G = np.concatenate([Gpa, Gp], axis=0).astype(np.float32)   # [128, H]

    return _head(G, pre["cnt"], inputs)


# revision 10
# speedup vs baseline: 3.4880x; 1.1011x over previous
"""Trainium2 Bass kernel for nn_CryptoGNN (2-layer GCN + pooled heads).

Math (same collapse as validated baseline):
  With A = normalized adjacency (incl. self loops), P = [B,N] pooling,
  u[d] = sum_{s->d} dis[s]x[s] + dis[d]x[d],  zhat = u@W1 + sq*b1,
  h1 = dis*relu(zhat) = dis*relu(zhat);  relu commutes with dis>0 so
  G = [PA;P]^T-pooled h1 uses papt columns pre-scaled by dis and
  h1hat = relu(zhat) on device.  Layer 2 + heads collapse to the host
  (tiny [64,*] math).

Per-core device pipeline (8-way node sharding, 12544 dst nodes/core):
  1. bf16 compacted src table DMA ([128, TW]: 8 banks x 6 feature rows,
     only srcs with >=1 edge into this core; all 128 rows host-written
     so no SBUF garbage can reach the PE - 0*NaN != 0 on TRN2 PE)
  2. expand bf16->fp32 split across Act/DVE (gather needs 4B elems)
  3. one GPSIMD ap_gather of all dst-sorted per-bank edge streams
  4. per dst-chunk (7 chunks x 1792 dsts): fp32 prefix scan (DVE),
     boundary ap_gather (GPSIMD), shifted diff -> dt bf16 (DVE 2x mode)
  5. per node tile: z = dt_t^T @ selW + aug_t^T @ w1aug   (bf16 PE,
     selW = bank-scattered W1 rows; fold+mm1 fused, no PSUM->SBUF hop)
  6. relu -> h1 bf16 (Act), G^T += h1_t^T @ papt_t accumulated in one
     [128, 80] PSUM across all 98 tiles; single gout DMA.
Host sums the 8 partial G^T and runs the small head in numpy.
"""

import sys

if "/opt/trn_rl_repo" not in sys.path:
    sys.path.insert(0, "/opt/trn_rl_repo")

import numpy as np
import ml_dtypes

N = 100000
E = 600000
B = 64
IN = 6
H = 128
S = 16

NG = 8                    # banks (= src chunks) and cores
NS = 12544                # nodes per core shard (98*128)
NPAD = NS * NG            # 100352
NT = 98                   # node tiles per shard
# dst chunks per core, in node tiles; last chunk small to shrink the tail
TCH = (15, 15, 15, 15, 15, 15, 8)
C = len(TCH)
NDCS = tuple(t * 128 for t in TCH)            # dsts per chunk
# boundary widths: +32 keeps int16 idx slice offsets 4-byte aligned
NBCS = tuple(n + 32 for n in NDCS)
DOFF = tuple(int(x) for x in np.concatenate([[0], np.cumsum(NDCS)]))
BOFF = tuple(int(x) for x in np.concatenate([[0], np.cumsum(NBCS)]))
NBT = BOFF[-1]
PCOL = 80                 # papt columns: 64 PA + <=16 local P
P128 = 128

_compiled = {}


def _build_nc(TW, JWS):
    import concourse.bacc as bacc
    import concourse.mybir as mybir
    from concourse import tile

    f32 = mybir.dt.float32
    bf16 = mybir.dt.bfloat16
    i16 = mybir.dt.int16

    JWT = sum(JWS)
    OFF = np.concatenate([[0], np.cumsum(JWS)]).astype(int)

    nc = bacc.Bacc("TRN2", target_bir_lowering=False, debug=False)

    xt = nc.declare_dram_parameter("xt", [P128, TW], bf16, isOutput=False)
    gidx = nc.declare_dram_parameter("gidx", [P128, JWT // 16], i16, isOutput=False)
    bidx = nc.declare_dram_parameter("bidx", [P128, NBT // 16], i16, isOutput=False)
    aug = nc.declare_dram_parameter("aug", [7, NS], bf16, isOutput=False)
    selw = nc.declare_dram_parameter("selw", [P128, H], bf16, isOutput=False)
    w1aug = nc.declare_dram_parameter("w1aug", [7, H], bf16, isOutput=False)
    papt = nc.declare_dram_parameter("papt", [NT * P128, PCOL], bf16, isOutput=False)
    gout = nc.declare_dram_parameter("gout", [P128, PCOL], f32, isOutput=True)

    with tile.TileContext(nc) as tc:
        with (
            tc.tile_pool(name="big", bufs=1) as big,
            tc.tile_pool(name="small", bufs=1) as small,
            tc.tile_pool(name="bndp", bufs=2) as bndp,
            tc.tile_pool(name="hbuf", bufs=3) as hbuf,
            tc.tile_pool(name="psz", bufs=2, space="PSUM") as pszp,
            tc.tile_pool(name="psG", bufs=1, space="PSUM") as psGp,
        ):
            # preload the activation-function table while DMAs run
            warm = small.tile([1, 2], f32)
            nc.vector.memset(warm[:], 0.0)
            nc.scalar.activation(out=warm[:], in_=warm[:],
                                 func=mybir.ActivationFunctionType.Copy)

            # ---------- loads (xt first: it gates the critical path) ----------
            TWH = (TW // 2 + 1) & ~1
            xt_t = big.tile([P128, TW], bf16, tag="xtb")
            nc.sync.dma_start(out=xt_t[:, 0:TWH], in_=xt[:, 0:TWH])
            nc.sync.dma_start(out=xt_t[:, TWH:TW], in_=xt[:, TWH:TW])
            gidx_t = small.tile([P128, JWT // 16], i16)
            nc.sync.dma_start(out=gidx_t[:], in_=gidx[:])
            bidx_t = small.tile([P128, NBT // 16], i16)
            nc.sync.dma_start(out=bidx_t[:], in_=bidx[:])
            selw_t = small.tile([P128, H], bf16)
            nc.sync.dma_start(out=selw_t[:], in_=selw[:])
            w1_t = small.tile([7, H], bf16)
            nc.sync.dma_start(out=w1_t[:], in_=w1aug[:])
            aug_t = small.tile([7, NS], bf16)
            nc.sync.dma_start(out=aug_t[:], in_=aug[:])
            papt_t = big.tile([P128, NT * PCOL], bf16, tag="papt")
            for c in range(C):
                nc.sync.dma_start(
                    out=papt_t[:, (DOFF[c] // 128) * PCOL:
                               (DOFF[c + 1] // 128) * PCOL].rearrange(
                        "p (u j) -> p u j", j=PCOL
                    ),
                    in_=papt[DOFF[c] : DOFF[c + 1], :].rearrange(
                        "(u p) j -> p u j", p=P128
                    ),
                )

            # ---------- expand table to fp32 (split Act / DVE, per DMA half) ----
            table = big.tile([P128, TW], f32, tag="table")
            # balance: Act 0.833 ns/el vs DVE 0.521 ns/el -> Act share 0.385
            for h0, h1e in ((0, TWH), (TWH, TW)):
                XA = (h0 + int((h1e - h0) * 0.385)) & ~1
                nc.scalar.activation(
                    out=table[:, h0:XA], in_=xt_t[:, h0:XA],
                    func=mybir.ActivationFunctionType.Copy,
                )
                nc.vector.tensor_copy(out=table[:, XA:h1e], in_=xt_t[:, XA:h1e])

            # ---------- gather all streams ----------
            gath = big.tile([P128, JWT], f32, tag="gath")
            nc.gpsimd.ap_gather(
                out_ap=gath[:], in_ap=table[:], idxs_ap=gidx_t[:],
                channels=P128, num_elems=TW, d=1, num_idxs=JWT,
            )

            # ---------- per-chunk scan (in place) ----------
            for c in range(C):
                o0, o1 = int(OFF[c]), int(OFF[c + 1])
                nc.vector.tensor_tensor_scan(
                    out=gath[:, o0:o1], data0=gath[:, o0:o1], data1=gath[:, o0:o1],
                    initial=0.0, op0=mybir.AluOpType.add,
                    op1=mybir.AluOpType.bypass,
                )

            # ---------- per-chunk boundary gather ----------
            bnds = []
            for c in range(C):
                o0, o1 = int(OFF[c]), int(OFF[c + 1])
                bnd = bndp.tile([P128, NBCS[c]], f32, tag=f"bnd{c % 2}")
                nc.gpsimd.ap_gather(
                    out_ap=bnd[:], in_ap=gath[:, o0:o1],
                    idxs_ap=bidx_t[:, BOFF[c] // 16:BOFF[c + 1] // 16],
                    channels=P128, num_elems=int(JWS[c]), d=1, num_idxs=NBCS[c],
                )
                bnds.append(bnd)

            # ---------- per-chunk diff -> dt (bf16) ----------
            dt = big.tile([P128, NS], bf16, tag="dt")
            for c in range(C):
                d0, nd = DOFF[c], NDCS[c]
                nc.vector.tensor_tensor(
                    out=dt[:, d0:d0 + nd],
                    in0=bnds[c][:, 1:1 + nd], in1=bnds[c][:, 0:nd],
                    op=mybir.AluOpType.subtract,
                )

            # ---------- phase B: z -> relu -> G (sw-pipelined batches) ----------
            G_ps = psGp.tile([P128, PCOL], f32, tag="G")
            QB = 8
            batches = []
            for c in range(C):
                t = DOFF[c] // 128
                left = TCH[c]
                while left > 0:
                    sz = min(QB, left)
                    batches.append((t, sz))
                    t += sz
                    left -= sz

            def z_mms(t0, m, ps):
                for u in range(m):
                    n0 = (t0 + u) * P128
                    nc.tensor.matmul(
                        out=ps[:, u * H:(u + 1) * H],
                        lhsT=dt[:, n0:n0 + P128], rhs=selw_t[:],
                        start=True, stop=False,
                    )
                    nc.tensor.matmul(
                        out=ps[:, u * H:(u + 1) * H],
                        lhsT=aug_t[:, n0:n0 + P128], rhs=w1_t[:],
                        start=False, stop=True,
                    )

            def g_mms(t0, m, h1):
                for u in range(m):
                    t = t0 + u
                    nc.tensor.matmul(
                        out=G_ps[:],
                        lhsT=h1[:, u * H:(u + 1) * H],
                        rhs=papt_t[:, t * PCOL:(t + 1) * PCOL],
                        start=(t == 0), stop=(t == NT - 1),
                    )

            prev = None
            for bi, (t0, m) in enumerate(batches):
                ps = pszp.tile([P128, QB * H], f32, tag="z")
                z_mms(t0, m, ps)
                h1 = hbuf.tile([P128, QB * H], bf16, tag="h1")
                nc.scalar.activation(
                    out=h1[:, :m * H], in_=ps[:, :m * H],
                    func=mybir.ActivationFunctionType.Relu,
                )
                if prev is not None:
                    g_mms(*prev)
                prev = (t0, m, h1)
            g_mms(*prev)

            G_sb = small.tile([P128, PCOL], f32)
            nc.vector.tensor_copy(out=G_sb[:], in_=G_ps[:])
            nc.sync.dma_start(out=gout[:], in_=G_sb[:])

    nc.compile()
    return nc


def _preprocess(x, edge_index, batch_idx):
    """Integer/structure preprocessing -> per-core device inputs."""
    src = np.asarray(edge_index[0], dtype=np.int64)
    dst = np.asarray(edge_index[1], dtype=np.int64)

    deg = (np.bincount(dst, minlength=N) + 1).astype(np.float32)
    dis = (1.0 / np.sqrt(deg)).astype(np.float32)
    sq = np.sqrt(deg).astype(np.float32)
    dis_pad = np.zeros(NPAD, np.float32)
    dis_pad[:N] = dis
    sq_pad = np.zeros(NPAD, np.float32)
    sq_pad[:N] = sq

    bi = np.asarray(batch_idx, dtype=np.int64)
    cnt = np.bincount(bi, minlength=B).astype(np.float32)

    x_np = np.asarray(x, dtype=np.float32)
    x_pad = np.zeros((NPAD, IN), np.float32)
    x_pad[:N] = x_np
    disx = x_pad * dis_pad[:, None]          # [NPAD, 6]

    # ---- pooling matrices (dense PA = P @ A) ----
    loop = np.arange(N, dtype=np.int64)
    src2 = np.concatenate([src, loop])
    dst2 = np.concatenate([dst, loop])
    w = (dis[src2] * dis[dst2]).astype(np.float64)
    flat = bi[dst2] * NPAD + src2
    PA = np.bincount(flat, weights=w, minlength=B * NPAD).reshape(B, NPAD)
    PA = PA.astype(np.float32)
    Pm = np.zeros((B, NPAD), np.float32)
    Pm[bi, np.arange(N)] = 1.0
    papt_full = (np.concatenate([PA, Pm], axis=0) * dis_pad[None, :]).T  # [NPAD,128]

    # graph span per core (for the P columns)
    first_graph = np.zeros(NG, np.int64)
    span = np.zeros(NG, np.int64)
    for k in range(NG):
        lo, hi = k * NS, min((k + 1) * NS, N)
        if lo >= N:
            first_graph[k] = B - 1
            span[k] = 1
            continue
        gset = bi[lo:hi]
        first_graph[k] = gset[0]
        span[k] = gset[-1] - gset[0] + 1
        assert span[k] <= PCOL - B, f"graph span {span[k]} > {PCOL - B}"

    # ---- per (core, bank) compacted streams ----
    core = dst // NS
    bank = src // NS
    src_local = src - bank * NS
    dst_local = dst - core * NS
    chunk = np.searchsorted(np.asarray(DOFF[1:]), dst_local, side="right")
    key = ((core * NG + bank) * C + chunk) * NS + dst_local
    order = np.argsort(key, kind="stable")
    core_s = core[order]
    bank_s = bank[order]
    chunk_s = chunk[order]
    srcl_s = src_local[order]
    dstl_s = dst_local[order]

    cell = (core_s * NG + bank_s) * C + chunk_s
    cellcnt = np.bincount(cell, minlength=NG * NG * C)
    cell_starts = np.zeros(NG * NG * C + 1, np.int64)
    np.cumsum(cellcnt, out=cell_starts[1:])

    # compact column maps per (core, bank)
    colmaps = {}
    ncols = np.zeros((NG, NG), np.int64)
    for k in range(NG):
        for g in range(NG):
            s0 = cell_starts[(k * NG + g) * C]
            s1 = cell_starts[(k * NG + g + 1) * C]
            uniq = np.unique(srcl_s[s0:s1])
            colmaps[(k, g)] = uniq
            ncols[k, g] = len(uniq)
    TW = int(ncols.max()) + 1
    TW = (TW + 15) & ~15

    # per-chunk stream widths (shared across cores for one compiled NEFF)
    cc = cellcnt.reshape(NG, NG, C)
    JWS = []
    for c in range(C):
        m = int(cc[:, :, c].max())
        JWS.append(((m + 1 + 15) // 16) * 16)
    JWT = sum(JWS)
    OFF = np.concatenate([[0], np.cumsum(JWS)]).astype(int)

    # build tables / idx arrays per core
    xt_all = np.zeros((NG, P128, TW), ml_dtypes.bfloat16)
    gidx_all = np.zeros((NG, P128, JWT // 16), np.int16)
    bidx_all = np.zeros((NG, P128, NBT // 16), np.int16)

    for k in range(NG):
        for g in range(NG):
            uniq = colmaps[(k, g)]
            n0 = g * NS
            xt_all[k, 16 * g:16 * g + 6, 1:1 + len(uniq)] = (
                disx[n0 + uniq].T.astype(ml_dtypes.bfloat16)
            )
            # remap this (core, bank)'s stream srcs to compact cols
            s0 = cell_starts[(k * NG + g) * C]
            s1 = cell_starts[(k * NG + g + 1) * C]
            comp = np.searchsorted(uniq, srcl_s[s0:s1]) + 1

            for c in range(C):
                c0 = cell_starts[(k * NG + g) * C + c]
                c1 = cell_starts[(k * NG + g) * C + c + 1]
                ncell = c1 - c0
                stream = np.zeros(JWS[c], np.int64)
                stream[1:1 + ncell] = comp[c0 - s0:c1 - s0]
                blk = stream.reshape(JWS[c] // 16, 16).T.astype(np.int16)
                gidx_all[k, 16 * g:16 * (g + 1), OFF[c] // 16:OFF[c + 1] // 16] = blk

                nd, nb = NDCS[c], NBCS[c]
                dloc = dstl_s[c0:c1] - DOFF[c]
                cnts = np.bincount(dloc, minlength=nd)
                blist = np.zeros(nb, np.int64)
                np.cumsum(cnts, out=blist[1:1 + nd])
                blist[1 + nd:] = blist[nd]
                bblk = blist.reshape(nb // 16, 16).T.astype(np.int16)
                bidx_all[k, 16 * g:16 * (g + 1),
                         BOFF[c] // 16:BOFF[c + 1] // 16] = bblk

    # aug rows: 0-5 dis*x own chunk, 6 sq
    aug_all = np.zeros((NG, 7, NS), ml_dtypes.bfloat16)
    for k in range(NG):
        n0 = k * NS
        aug_all[k, 0:6] = disx[n0:n0 + NS].T.astype(ml_dtypes.bfloat16)
        aug_all[k, 6] = sq_pad[n0:n0 + NS].astype(ml_dtypes.bfloat16)

    # papt per core: 64 PA cols + local P cols, blocked [NT*128, PCOL]
    papt_all = np.zeros((NG, NT * P128, PCOL), ml_dtypes.bfloat16)
    for k in range(NG):
        n0 = k * NS
        pk = np.zeros((NS, PCOL), np.float32)
        pk[:, :B] = papt_full[n0:n0 + NS, :B]
        b0, sp = first_graph[k], span[k]
        pk[:, B:B + sp] = papt_full[n0:n0 + NS, B + b0:B + b0 + sp]
        papt_all[k] = pk.astype(ml_dtypes.bfloat16)

    return {
        "JW": (TW, tuple(JWS)),
        "TW": TW,
        "JWS": JWS,
        "xt_all": xt_all,
        "gidx_all": gidx_all,
        "bidx_all": bidx_all,
        "aug_all": aug_all,
        "papt_all": papt_all,
        "first_graph": first_graph,
        "span": span,
        "cnt": cnt,
    }


def _head(G, cnt, inputs):
    f = np.float32
    W2 = np.asarray(inputs["W2"], f)
    b2 = np.asarray(inputs["b2"], f)
    Wg = np.asarray(inputs["Wg"], f)
    bg = np.asarray(inputs["bg"], f)
    Et = np.asarray(inputs["Et"], f)
    Ek = np.asarray(inputs["Ek"], f)
    Ev = np.asarray(inputs["Ev"], f)
    Wp = np.asarray(inputs["Wp"], f)
    bp = np.asarray(inputs["bp"], f)
    Ekid = np.asarray(inputs["Ekid"], f)
    Wc = np.asarray(inputs["Wc"], f)
    bc = np.asarray(inputs["bc"], f)
    Wl = np.asarray(inputs["Wl"], f)
    bl = np.asarray(inputs["bl"], f)
    Wm1 = np.asarray(inputs["Wm1"], f)
    bm1 = np.asarray(inputs["bm1"], f)
    Wm2 = np.asarray(inputs["Wm2"], f)
    bm2 = np.asarray(inputs["bm2"], f)
    st = np.asarray(inputs["sol_type_idx"], np.int64)
    sk = np.asarray(inputs["sol_key_idx"], np.int64)
    sv = np.asarray(inputs["sol_val_idx"], np.int64)
    kid = np.asarray(inputs["kernel_id"], np.int64)
    cond = np.asarray(inputs["cond_vec"], f)
    loc = np.asarray(inputs["local_feats"], f)

    relu = lambda a: np.maximum(a, 0.0).astype(f)

    Ph2 = G[:B] @ W2 + cnt[:, None] * b2[None, :] + G[B:]
    g = (Ph2 / np.maximum(cnt, 1.0)[:, None]) @ Wg + bg

    seq_mean = np.concatenate(
        [Et[st].mean(axis=1), Ek[sk].mean(axis=1), Ev[sv].mean(axis=1)], axis=-1
    ).astype(f)
    p = relu(seq_mean @ Wp + bp)
    kvec = Ekid[kid]
    c = relu(cond @ Wc + bc)
    l = relu(loc @ Wl + bl)
    xf = np.concatenate([g, p, kvec, c, l], axis=1).astype(f)
    return (relu(xf @ Wm1 + bm1) @ Wm2 + bm2).astype(f)


def kernel(**inputs) -> np.ndarray:
    from concourse.bass_utils import run_bass_kernel_spmd

    pre = _preprocess(inputs["x"], inputs["edge_index"], inputs["batch_idx"])
    sig = pre["JW"]
    if sig not in _compiled:
        _compiled[sig] = _build_nc(pre["TW"], tuple(pre["JWS"]))
    nc = _compiled[sig]

    W1 = np.asarray(inputs["W1"], np.float32)
    b1 = np.asarray(inputs["b1"], np.float32)
    w1aug = np.concatenate([W1, b1[None, :]], axis=0).astype(ml_dtypes.bfloat16)
    selw = np.zeros((P128, H), ml_dtypes.bfloat16)
    for g in range(NG):
        selw[16 * g:16 * g + 6] = W1.astype(ml_dtypes.bfloat16)

    in_maps = []
    for k in range(NG):
        in_maps.append({
            "xt": pre["xt_all"][k],
            "gidx": pre["gidx_all"][k],
            "bidx": pre["bidx_all"][k],
            "aug": pre["aug_all"][k],
            "selw": selw,
            "w1aug": w1aug,
            "papt": pre["papt_all"][k],
        })

    res = run_bass_kernel_spmd(nc, in_maps, core_ids=list(range(NG)))

    Gpa = np.zeros((B, H), np.float64)
    Gp = np.zeros((B, H), np.float64)
    for k, r in enumerate(res.results):
        gt = r["gout"].astype(np.float64)      # [128 f, 80 c]
        Gpa += gt[:, :B].T
        b0, sp = pre["first_graph"][k], pre["span"][k]
        Gp[b0:b0 + sp] += gt[:, B:B + sp].T
    G = np.concatenate([Gpa, Gp], axis=0).astype(np.float32)   # [128, H]

    return _head(G, pre["cnt"], inputs)
